# revision 10
# baseline (speedup 1.0000x reference)
"""GATv2 layer on 8 Trainium2 NeuronCores (Bass/Tile).

Sharding (edge-parallel by destination range): device d owns dst nodes
[d*6250, (d+1)*6250). Host sorts edges by dst, routes each edge to its
dst's device, and ships PRE-TRANSPOSED gathered node features
(feature-major [128, n_slots] fp16, src block | dst block per group of
512 edges) so every load is a plain contiguous-per-partition DMA (2 KB
lines) instead of a transpose-DMA (256 B packets).

Per group of 512 edges (4 tiles x 128):
  PE    z0T [hf,512] = WsrcT.T @ seT + WdstT.T @ deT   (weights stationary)
  ACT   tT = LeakyReLU(z0T)            (exact: alpha=0.2)
  PE    scE [128,t,4] = tSB_tile.T @ attn_sel  (edge-major score: stationary
                                        = data slice, moving = 4-col attn_sel)
  ACT   exS = Exp(scE)                 (no max-shift: scores O(1), softmax
                                        shift-invariant, matches reference)
  PE    fs [128,t,HF] = seT_tile.T @ WsrcT            (edge-major)
  POOL  onehot[e,n] = (drel[e,t] == iota[n])          (from 2B/edge drel)
  DVE   msgex = [fs * ex | ex]
  PE    acc [n, HF+4] += onehot.T @ msgex             (segment softmax sums
                                                       as PSUM matmuls)
Window tail: out = acc[:, :HF]/max(acc[:, HF:],eps) + res_win (residual
projections precomputed upfront from host-transposed window features).
"""
import sys
import numpy as np

sys.path.insert(0, "/opt/trn_rl_repo")

import concourse.bass as bass  # noqa: E402
import concourse.bacc as bacc  # noqa: E402
import concourse.tile as tile  # noqa: E402
from concourse import mybir  # noqa: E402
from concourse.bass_utils import run_bass_kernel_spmd  # noqa: E402

IN_FEATS = 128
N_HEADS = 4
OUT_FEATS = 32
HF = N_HEADS * OUT_FEATS  # 128
SLOPE = 0.2
P = 128
GRP = 4                              # tiles per group (512 edges)
NSPLIT = 4                           # parts for the big edge-feature input


def _set_sizes(n_nodes=50000, m=8):
    global N_NODES, M, NLOC, WIN, OUT_ROWS
    N_NODES = n_nodes
    M = m
    NLOC = N_NODES // M
    WIN = (NLOC + P - 1) // P
    OUT_ROWS = WIN * P


_set_sizes()

f16 = mybir.dt.float16
f32 = mybir.dt.float32

_prog_cache = {}


def _build_program(K: int, n_groups: int):
    nc = bacc.Bacc("TRN2", debug=False, num_devices=M)

    gpp = (n_groups + NSPLIT - 1) // NSPLIT   # groups per part
    part_cols = [2 * GRP * P * min(gpp, n_groups - i * gpp) for i in range(NSPLIT)]
    sedeT_p = [nc.dram_tensor(f"sedeT{i}", [P, part_cols[i]], f16,
                              kind="ExternalInput") for i in range(NSPLIT)]
    fwinT = nc.dram_tensor("fwinT", [P, OUT_ROWS], f16, kind="ExternalInput")
    wsrcT = nc.dram_tensor("wsrcT", [IN_FEATS, HF], f16, kind="ExternalInput")
    wdstT = nc.dram_tensor("wdstT", [IN_FEATS, HF], f16, kind="ExternalInput")
    wresT = nc.dram_tensor("wresT", [IN_FEATS, HF], f16, kind="ExternalInput")
    attn_sel = nc.dram_tensor("attn_sel", [HF, N_HEADS], f16, kind="ExternalInput")
    iota_row = nc.dram_tensor("iota_row", [P, P], f16, kind="ExternalInput")
    drel_in = nc.dram_tensor("drel", [P, n_groups * GRP], f16, kind="ExternalInput")
    out_d = nc.dram_tensor("out_d", [OUT_ROWS, HF], f32, kind="ExternalOutput")

    with tile.TileContext(nc) as tc:
        with tc.tile_pool(name="const", bufs=1) as cpool:
            ws = cpool.tile([IN_FEATS, HF], f16, tag="ws")
            wd = cpool.tile([IN_FEATS, HF], f16, tag="wd")
            wr = cpool.tile([IN_FEATS, HF], f16, tag="wr")
            asel = cpool.tile([HF, N_HEADS], f16, tag="asel")
            iota = cpool.tile([P, P], f16, tag="iota")
            drel = cpool.tile([P, n_groups, GRP], f16, tag="drel")
            fwin = cpool.tile([P, OUT_ROWS], f16, tag="fwin")
            res_sb = cpool.tile([P, WIN, HF], f32, tag="res")
            nc.sync.dma_start(ws[:], wsrcT[:])
            nc.sync.dma_start(wd[:], wdstT[:])
            nc.sync.dma_start(wr[:], wresT[:])
            nc.sync.dma_start(asel[:], attn_sel[:])
            nc.sync.dma_start(iota[:], iota_row[:])
            nc.sync.dma_start(
                drel[:], drel_in[:].rearrange("p (g t) -> p g t", t=GRP))
            nc.sync.dma_start(fwin[:], fwinT[:])

            with tc.tile_pool(name="pb", bufs=3) as pb, \
                 tc.tile_pool(name="pw", bufs=2) as pw, \
                 tc.tile_pool(name="ps_z", bufs=2, space="PSUM") as ps_z, \
                 tc.tile_pool(name="ps_fs", bufs=2, space="PSUM") as ps_fs, \
                 tc.tile_pool(name="ps_s", bufs=2, space="PSUM") as ps_s, \
                 tc.tile_pool(name="ps_acc", bufs=2, space="PSUM") as ps_acc:

                # residual projections for all windows, upfront (PSUM borrowed
                # from the fs pool — only the [:, 0, :] slice is used)
                for w in range(WIN):
                    sl = slice(w * P, (w + 1) * P)
                    res_ps = ps_fs.tile([P, GRP, HF], f32, tag="fs")
                    nc.tensor.matmul(res_ps[:, 0, :], lhsT=fwin[:, sl],
                                     rhs=wr[:], start=True, stop=True)
                    nc.scalar.activation(
                        out=res_sb[:, w, :], in_=res_ps[:, 0, :],
                        func=mybir.ActivationFunctionType.Copy)

                acc = None
                for g in range(n_groups):
                    pi, go = divmod(g, gpp)
                    cols = 2 * GRP * P
                    sdT = pb.tile([P, cols], f16, tag="sdT")
                    nc.sync.dma_start(
                        sdT[:], sedeT_p[pi][:, cols * go:cols * (go + 1)])
                    seT = sdT[:, 0:GRP * P]
                    deT = sdT[:, GRP * P:2 * GRP * P]

                    # score pipeline (feature-major)
                    z0 = ps_z.tile([P, GRP * P], f32, tag="z0")
                    nc.tensor.matmul(z0[:], lhsT=ws[:], rhs=seT,
                                     start=True, stop=False)
                    nc.tensor.matmul(z0[:], lhsT=wd[:], rhs=deT,
                                     start=False, stop=True)
                    tSB = pw.tile([P, GRP * P], f16, tag="tSB")
                    nc.scalar.activation(
                        out=tSB[:], in_=z0[:],
                        func=mybir.ActivationFunctionType.Lrelu, alpha=SLOPE)
                    # edge-major: score (4-col moving matmul), fs, onehot
                    scE = ps_s.tile([P, GRP, N_HEADS], f32, tag="scE")
                    fs_ps = ps_fs.tile([P, GRP, HF], f32, tag="fs")
                    for t in range(GRP):
                        sl = slice(t * P, (t + 1) * P)
                        nc.tensor.matmul(scE[:, t, :], lhsT=tSB[:, sl],
                                         rhs=asel[:], start=True, stop=True)
                        nc.tensor.matmul(fs_ps[:, t, :], lhsT=seT[:, sl],
                                         rhs=ws[:], start=True, stop=True)
                    exS = pw.tile([P, GRP, N_HEADS], f32, tag="exS")
                    nc.scalar.activation(out=exS[:], in_=scE[:],
                                         func=mybir.ActivationFunctionType.Exp)
                    oh = pw.tile([P, GRP, P], f16, tag="oh")
                    nc.vector.tensor_tensor(
                        out=oh[:],
                        in0=drel[:, g, :][:, :, None].to_broadcast([P, GRP, P]),
                        in1=iota[:][:, None, :].to_broadcast([P, GRP, P]),
                        op=mybir.AluOpType.is_equal)

                    msgex = pw.tile([P, GRP, HF + N_HEADS], f16, tag="msgex")
                    nc.vector.tensor_tensor(
                        out=msgex[:, :, 0:HF].rearrange(
                            "p t (h f) -> p t h f", h=N_HEADS),
                        in0=fs_ps[:].rearrange("p t (h f) -> p t h f", h=N_HEADS),
                        in1=exS[:][:, :, :, None]
                            .to_broadcast([P, GRP, N_HEADS, OUT_FEATS]),
                        op=mybir.AluOpType.mult)
                    nc.scalar.activation(
                        out=msgex[:, :, HF:HF + N_HEADS], in_=exS[:],
                        func=mybir.ActivationFunctionType.Copy)

                    n_slots_t = GRP * n_groups
                    for t in range(GRP):
                        tau = GRP * g + t
                        # trailing dummy tiles (drel=-1 -> zero one-hot) fold
                        # into the last window's accumulation group
                        w = min(tau // K, WIN - 1)
                        k = tau - w * K
                        last_k = (K - 1) if w < WIN - 1 else (n_slots_t - 1 - w * K)
                        if k == 0:
                            acc = ps_acc.tile([P, HF + N_HEADS], f32, tag="acc")
                        nc.tensor.matmul(acc[:], lhsT=oh[:, t, :],
                                         rhs=msgex[:, t, :],
                                         start=(k == 0), stop=(k == last_k))
                        if k == last_k:
                            den = pb.tile([P, N_HEADS], f32, tag="den")
                            nc.vector.tensor_scalar(
                                out=den[:], in0=acc[:, HF:HF + N_HEADS],
                                scalar1=1e-30, scalar2=None,
                                op0=mybir.AluOpType.max)
                            rec = pb.tile([P, N_HEADS], f32, tag="rec")
                            nc.vector.reciprocal(out=rec[:], in_=den[:])
                            osb = pb.tile([P, HF], f32, tag="osb")
                            for h in range(N_HEADS):
                                sl = slice(h * OUT_FEATS, (h + 1) * OUT_FEATS)
                                nc.vector.scalar_tensor_tensor(
                                    out=osb[:, sl], in0=acc[:, sl],
                                    scalar=rec[:, h:h + 1],
                                    in1=res_sb[:, w, sl],
                                    op0=mybir.AluOpType.mult,
                                    op1=mybir.AluOpType.add)
                            nc.sync.dma_start(
                                out_d[w * P:(w + 1) * P, :], osb[:])

    nc.compile()
    return nc


def _preprocess(feat, W_src, b_src, W_dst, b_dst, attn_e, W_res, b_res, src, dst):
    """Host-side sharding: sort edges by dst, build per-core inputs."""
    feat = np.asarray(feat, dtype=np.float32)
    b_src = np.asarray(b_src, np.float32)
    b_dst = np.asarray(b_dst, np.float32)
    b_res = np.asarray(b_res, np.float32)
    assert not (b_src.any() or b_dst.any() or b_res.any()), \
        "nonzero biases not supported by this kernel build"
    src = np.asarray(src, dtype=np.int64)
    dst = np.asarray(dst, dtype=np.int64)

    order = np.argsort(dst, kind="stable")
    src_s = src[order]
    dst_s = dst[order]

    dev_bounds = np.searchsorted(dst_s, np.arange(M + 1) * NLOC)
    per_dev = []
    K = 1
    for d in range(M):
        e0, e1 = dev_bounds[d], dev_bounds[d + 1]
        sd = src_s[e0:e1]
        dd = dst_s[e0:e1] - d * NLOC
        w = dd // P
        starts = np.searchsorted(dd, np.arange(WIN) * P)
        r = np.arange(len(dd)) - starts[w]
        counts = np.diff(np.searchsorted(dd, np.arange(0, WIN * P + P, P)))
        if len(dd):
            K = max(K, int((counts.max() + P - 1) // P))
        per_dev.append((sd, dd, w, r))

    n_tiles = WIN * K
    n_groups = (n_tiles + GRP - 1) // GRP
    n_slots = GRP * n_groups * P

    feat16 = feat.astype(np.float16)
    feat16T = np.ascontiguousarray(feat16.T)          # [128, N]
    se_ids = np.zeros((M, n_slots), dtype=np.int64)
    de_ids = np.zeros((M, n_slots), dtype=np.int64)
    drel_all = np.full((M, n_groups, P, GRP), -1.0, dtype=np.float16)

    for d in range(M):
        sd, dd, w, r = per_dev[d]
        if not len(dd):
            continue
        tau = w * K + r // P
        p = r % P
        slot = tau * P + p            # tile-major, partition-minor
        se_ids[d, slot] = sd
        de_ids[d, slot] = dd + d * NLOC
        g = tau // GRP
        t = tau % GRP
        drel_all[d, g, p, t] = (dd - w * P).astype(np.float16)

    attn_f = np.asarray(attn_e, np.float32).reshape(HF)
    attn_sel = np.zeros((HF, N_HEADS), dtype=np.float16)
    attn_sel[np.arange(HF), np.arange(HF) // OUT_FEATS] = attn_f.astype(np.float16)

    cst = {
        "wsrcT": np.ascontiguousarray(
            np.asarray(W_src, np.float32).T).astype(np.float16),
        "wdstT": np.ascontiguousarray(
            np.asarray(W_dst, np.float32).T).astype(np.float16),
        "wresT": np.ascontiguousarray(
            np.asarray(W_res, np.float32).T).astype(np.float16),
        "attn_sel": attn_sel,
        "iota_row": np.tile(np.arange(P, dtype=np.float16)[None, :], (P, 1)),
    }

    gpp = (n_groups + NSPLIT - 1) // NSPLIT
    in_maps = []
    for d in range(M):
        m = dict(cst)
        # feature-major gathered features: per group [se 512 | de 512]
        fseT = feat16T[:, se_ids[d]]                    # [128, n_slots]
        fdeT = feat16T[:, de_ids[d]]
        sede = np.empty((P, n_groups, 2 * GRP * P), dtype=np.float16)
        sede[:, :, 0:GRP * P] = fseT.reshape(P, n_groups, GRP * P)
        sede[:, :, GRP * P:] = fdeT.reshape(P, n_groups, GRP * P)
        for i in range(NSPLIT):
            g0 = gpp * i
            g1 = min(gpp * (i + 1), n_groups)
            m[f"sedeT{i}"] = np.ascontiguousarray(
                sede[:, g0:g1, :].reshape(P, -1))
        fwin = np.zeros((OUT_ROWS, IN_FEATS), dtype=np.float16)
        fwin[:NLOC] = feat16[d * NLOC:(d + 1) * NLOC]
        m["fwinT"] = np.ascontiguousarray(fwin.T)
        m["drel"] = np.ascontiguousarray(
            drel_all[d].transpose(1, 0, 2).reshape(P, n_groups * GRP))
        in_maps.append(m)
    return K, n_groups, in_maps


def kernel(feat, W_src, b_src, W_dst, b_dst, attn_e, W_res, b_res, src, dst,
           _trace=False, _trace_kwargs=None):
    K, n_groups, in_maps = _preprocess(feat, W_src, b_src, W_dst, b_dst,
                                       attn_e, W_res, b_res, src, dst)
    key = (K, n_groups)
    if key not in _prog_cache:
        _prog_cache[key] = _build_program(K, n_groups)
    nc = _prog_cache[key]

    kw = {}
    if _trace:
        kw = dict(trace=True, trace_kwargs=_trace_kwargs or {})
    res = run_bass_kernel_spmd(nc, in_maps, core_ids=list(range(M)), **kw)
    outs = [res.results[d]["out_d"][:NLOC] for d in range(M)]
    full = np.concatenate(outs, axis=0).reshape(N_NODES, N_HEADS, OUT_FEATS)
    kernel._last_results = res
    kernel._last_cfg = (K, n_groups)
    return full


# revision 18
# speedup vs baseline: 1.1504x; 1.1504x over previous
"""GATv2 layer on 8 Trainium2 NeuronCores (Bass/Tile).

Sharding (edge-parallel by destination range): device d owns dst nodes
[d*6250, (d+1)*6250). Host sorts edges by dst, routes each edge to its
dst's device, and ships PRE-TRANSPOSED gathered node features
(feature-major [128, n_slots] fp16, src block | dst block per group of
512 edges) so every load is a plain contiguous-per-partition DMA (2 KB
lines) instead of a transpose-DMA (256 B packets).

Per group of 512 edges (4 tiles x 128):
  PE    z0T [hf,512] = WsrcT.T @ seT + WdstT.T @ deT   (weights stationary)
  ACT   ay = |z0T|   (Abs shares the resident exp_and_others ACT table with
                      Exp/Copy -- Lrelu would force a 1.3us table reload per
                      group. score = 0.6*sum attn*z0 + 0.4*sum attn*|z0|.)
  PE    scE [128,t,4] = seT_t.T @ Wu_src + deT_t.T @ Wu_dst   (linear u part)
                      + ay_t.T @ (0.4*attn_sel)               (abs part)
        (edge-major score: stationary = data slice, moving = 4-col weights)
  ACT   exS = Exp(scE)                 (no max-shift: scores O(1), softmax
                                        shift-invariant, matches reference)
  PE    fs [128,t,HF] = seT_tile.T @ WsrcT            (edge-major)
  POOL  onehot[e,n] = (drel[e,t] == iota[n])          (from 2B/edge drel)
  DVE   msgex = [fs * ex | ex]
  PE    acc [n, HF+4] += onehot.T @ msgex             (segment softmax sums
                                                       as PSUM matmuls)
Window tail: out = acc[:, :HF]/max(acc[:, HF:],eps) + res_win (residual
projections precomputed upfront from host-transposed window features).
"""
import sys
import numpy as np

sys.path.insert(0, "/opt/trn_rl_repo")

import concourse.bass as bass  # noqa: E402
import concourse.bacc as bacc  # noqa: E402
import concourse.tile as tile  # noqa: E402
from concourse import mybir  # noqa: E402
from concourse.bass_utils import run_bass_kernel_spmd  # noqa: E402

IN_FEATS = 128
N_HEADS = 4
OUT_FEATS = 32
HF = N_HEADS * OUT_FEATS  # 128
SLOPE = 0.2
P = 128
GRP = 4                              # tiles per group (512 edges)
NSPLIT = 4                           # parts for the big edge-feature input


def _set_sizes(n_nodes=50000, m=8):
    global N_NODES, M, NLOC, WIN, OUT_ROWS
    N_NODES = n_nodes
    M = m
    NLOC = N_NODES // M
    WIN = (NLOC + P - 1) // P
    OUT_ROWS = WIN * P


_set_sizes()

f16 = mybir.dt.float16
f32 = mybir.dt.float32

_prog_cache = {}


def _build_program(K: int, n_groups: int):
    nc = bacc.Bacc("TRN2", debug=False, num_devices=M)

    gpp = (n_groups + NSPLIT - 1) // NSPLIT   # groups per part
    part_cols = [2 * GRP * P * min(gpp, n_groups - i * gpp) for i in range(NSPLIT)]
    sedeT_p = [nc.dram_tensor(f"sedeT{i}", [P, part_cols[i]], f16,
                              kind="ExternalInput") for i in range(NSPLIT)]
    fwinT = nc.dram_tensor("fwinT", [P, OUT_ROWS], f16, kind="ExternalInput")
    wsrcT = nc.dram_tensor("wsrcT", [IN_FEATS, HF], f16, kind="ExternalInput")
    wdstT = nc.dram_tensor("wdstT", [IN_FEATS, HF], f16, kind="ExternalInput")
    wresT = nc.dram_tensor("wresT", [IN_FEATS, HF], f16, kind="ExternalInput")
    attn_sel = nc.dram_tensor("attn_sel", [HF, N_HEADS], f16, kind="ExternalInput")
    wu_src = nc.dram_tensor("wu_src", [IN_FEATS, N_HEADS], f16, kind="ExternalInput")
    wu_dst = nc.dram_tensor("wu_dst", [IN_FEATS, N_HEADS], f16, kind="ExternalInput")
    iota_row = nc.dram_tensor("iota_row", [P, P], f16, kind="ExternalInput")
    drel_in = nc.dram_tensor("drel", [P, n_groups * GRP], f32, kind="ExternalInput")
    out_d = nc.dram_tensor("out_d", [OUT_ROWS, HF], f32, kind="ExternalOutput")

    with tile.TileContext(nc) as tc:
        with tc.tile_pool(name="const", bufs=1) as cpool:
            ws = cpool.tile([IN_FEATS, HF], f16, tag="ws")
            wd = cpool.tile([IN_FEATS, HF], f16, tag="wd")
            wr = cpool.tile([IN_FEATS, HF], f16, tag="wr")
            asel = cpool.tile([HF, N_HEADS], f16, tag="asel")
            wus = cpool.tile([IN_FEATS, N_HEADS], f16, tag="wus")
            wud = cpool.tile([IN_FEATS, N_HEADS], f16, tag="wud")
            iota = cpool.tile([P, P], f16, tag="iota")
            drel = cpool.tile([P, n_groups, GRP], f32, tag="drel")
            fwin = cpool.tile([P, OUT_ROWS], f16, tag="fwin")
            res_sb = cpool.tile([P, WIN, HF], f32, tag="res")
            nc.sync.dma_start(ws[:], wsrcT[:])
            nc.sync.dma_start(wd[:], wdstT[:])
            nc.sync.dma_start(wr[:], wresT[:])
            nc.sync.dma_start(asel[:], attn_sel[:])
            nc.sync.dma_start(wus[:], wu_src[:])
            nc.sync.dma_start(wud[:], wu_dst[:])
            nc.sync.dma_start(iota[:], iota_row[:])
            nc.sync.dma_start(
                drel[:], drel_in[:].rearrange("p (g t) -> p g t", t=GRP))
            nc.sync.dma_start(fwin[:], fwinT[:])

            with tc.tile_pool(name="pb", bufs=3) as pb, \
                 tc.tile_pool(name="pw", bufs=2) as pw, \
                 tc.tile_pool(name="ps_z", bufs=2, space="PSUM") as ps_z, \
                 tc.tile_pool(name="ps_fs", bufs=2, space="PSUM") as ps_fs, \
                 tc.tile_pool(name="ps_s", bufs=2, space="PSUM") as ps_s, \
                 tc.tile_pool(name="ps_acc", bufs=2, space="PSUM") as ps_acc:

                # residual projections for all windows, upfront (PSUM borrowed
                # from the fs pool — only the [:, 0, :] slice is used)
                for w in range(WIN):
                    sl = slice(w * P, (w + 1) * P)
                    res_ps = ps_fs.tile([P, GRP, HF], f32, tag="fs")
                    nc.tensor.matmul(res_ps[:, 0, :], lhsT=fwin[:, sl],
                                     rhs=wr[:], start=True, stop=True)
                    nc.scalar.activation(
                        out=res_sb[:, w, :], in_=res_ps[:, 0, :],
                        func=mybir.ActivationFunctionType.Copy)

                acc = None
                for g in range(n_groups):
                    pi, go = divmod(g, gpp)
                    cols = 2 * GRP * P
                    sdT = pb.tile([P, cols], f16, tag="sdT")
                    nc.sync.dma_start(
                        sdT[:], sedeT_p[pi][:, cols * go:cols * (go + 1)])
                    seT = sdT[:, 0:GRP * P]
                    deT = sdT[:, GRP * P:2 * GRP * P]

                    # score pipeline (feature-major)
                    z0 = ps_z.tile([P, GRP * P], f32, tag="z0")
                    nc.tensor.matmul(z0[:], lhsT=ws[:], rhs=seT,
                                     start=True, stop=False)
                    nc.tensor.matmul(z0[:], lhsT=wd[:], rhs=deT,
                                     start=False, stop=True)
                    ay = pw.tile([P, GRP * P], f16, tag="ay")
                    nc.scalar.activation(
                        out=ay[:], in_=z0[:],
                        func=mybir.ActivationFunctionType.Abs)
                    # edge-major: score (4-col moving matmuls), fs, onehot
                    scE = ps_s.tile([P, GRP, N_HEADS], f32, tag="scE")
                    fs_ps = ps_fs.tile([P, GRP, HF], f32, tag="fs")
                    oh = pw.tile([P, GRP, P], f16, tag="oh")
                    for t in range(GRP):
                        sl = slice(t * P, (t + 1) * P)
                        nc.tensor.matmul(scE[:, t, :], lhsT=seT[:, sl],
                                         rhs=wus[:], start=True, stop=False)
                        nc.tensor.matmul(scE[:, t, :], lhsT=deT[:, sl],
                                         rhs=wud[:], start=False, stop=False)
                        nc.tensor.matmul(scE[:, t, :], lhsT=ay[:, sl],
                                         rhs=asel[:], start=False, stop=True)
                        nc.tensor.matmul(fs_ps[:, t, :], lhsT=seT[:, sl],
                                         rhs=ws[:], start=True, stop=True)
                        nc.vector.tensor_scalar(
                            out=oh[:, t, :], in0=iota[:],
                            scalar1=drel[:, g, t:t + 1], scalar2=None,
                            op0=mybir.AluOpType.is_equal)
                    exS = pw.tile([P, GRP, N_HEADS], f32, tag="exS")
                    nc.scalar.activation(out=exS[:], in_=scE[:],
                                         func=mybir.ActivationFunctionType.Exp)

                    msgex = pw.tile([P, GRP, HF + N_HEADS], f16, tag="msgex")
                    nc.vector.tensor_tensor(
                        out=msgex[:, :, 0:HF].rearrange(
                            "p t (h f) -> p t h f", h=N_HEADS),
                        in0=fs_ps[:].rearrange("p t (h f) -> p t h f", h=N_HEADS),
                        in1=exS[:][:, :, :, None]
                            .to_broadcast([P, GRP, N_HEADS, OUT_FEATS]),
                        op=mybir.AluOpType.mult)
                    nc.scalar.activation(
                        out=msgex[:, :, HF:HF + N_HEADS], in_=exS[:],
                        func=mybir.ActivationFunctionType.Copy)

                    n_slots_t = GRP * n_groups
                    for t in range(GRP):
                        tau = GRP * g + t
                        # trailing dummy tiles (drel=-1 -> zero one-hot) fold
                        # into the last window's accumulation group
                        w = min(tau // K, WIN - 1)
                        k = tau - w * K
                        last_k = (K - 1) if w < WIN - 1 else (n_slots_t - 1 - w * K)
                        if k == 0:
                            acc = ps_acc.tile([P, HF + N_HEADS], f32, tag="acc")
                        nc.tensor.matmul(acc[:], lhsT=oh[:, t, :],
                                         rhs=msgex[:, t, :],
                                         start=(k == 0), stop=(k == last_k))
                        if k == last_k:
                            den = pb.tile([P, N_HEADS], f32, tag="den")
                            nc.vector.tensor_scalar(
                                out=den[:], in0=acc[:, HF:HF + N_HEADS],
                                scalar1=1e-30, scalar2=None,
                                op0=mybir.AluOpType.max)
                            rec = pb.tile([P, N_HEADS], f32, tag="rec")
                            nc.vector.reciprocal(out=rec[:], in_=den[:])
                            osb = pb.tile([P, HF], f32, tag="osb")
                            for h in range(N_HEADS):
                                sl = slice(h * OUT_FEATS, (h + 1) * OUT_FEATS)
                                nc.vector.scalar_tensor_tensor(
                                    out=osb[:, sl], in0=acc[:, sl],
                                    scalar=rec[:, h:h + 1],
                                    in1=res_sb[:, w, sl],
                                    op0=mybir.AluOpType.mult,
                                    op1=mybir.AluOpType.add)
                            nc.sync.dma_start(
                                out_d[w * P:(w + 1) * P, :], osb[:])

    nc.compile()
    return nc


def _preprocess(feat, W_src, b_src, W_dst, b_dst, attn_e, W_res, b_res, src, dst):
    """Host-side sharding: sort edges by dst, build per-core inputs."""
    feat = np.asarray(feat, dtype=np.float32)
    b_src = np.asarray(b_src, np.float32)
    b_dst = np.asarray(b_dst, np.float32)
    b_res = np.asarray(b_res, np.float32)
    assert not (b_src.any() or b_dst.any() or b_res.any()), \
        "nonzero biases not supported by this kernel build"
    src = np.asarray(src, dtype=np.int64)
    dst = np.asarray(dst, dtype=np.int64)

    order = np.argsort(dst, kind="stable")
    src_s = src[order]
    dst_s = dst[order]

    dev_bounds = np.searchsorted(dst_s, np.arange(M + 1) * NLOC)
    per_dev = []
    K = 1
    for d in range(M):
        e0, e1 = dev_bounds[d], dev_bounds[d + 1]
        sd = src_s[e0:e1]
        dd = dst_s[e0:e1] - d * NLOC
        w = dd // P
        starts = np.searchsorted(dd, np.arange(WIN) * P)
        r = np.arange(len(dd)) - starts[w]
        counts = np.diff(np.searchsorted(dd, np.arange(0, WIN * P + P, P)))
        if len(dd):
            K = max(K, int((counts.max() + P - 1) // P))
        per_dev.append((sd, dd, w, r))

    n_tiles = WIN * K
    n_groups = (n_tiles + GRP - 1) // GRP
    n_slots = GRP * n_groups * P

    feat16 = feat.astype(np.float16)
    feat16T = np.ascontiguousarray(feat16.T)          # [128, N]
    se_ids = np.zeros((M, n_slots), dtype=np.int64)
    de_ids = np.zeros((M, n_slots), dtype=np.int64)
    drel_all = np.full((M, n_groups, P, GRP), -1.0, dtype=np.float16)

    for d in range(M):
        sd, dd, w, r = per_dev[d]
        if not len(dd):
            continue
        tau = w * K + r // P
        p = r % P
        slot = tau * P + p            # tile-major, partition-minor
        se_ids[d, slot] = sd
        de_ids[d, slot] = dd + d * NLOC
        g = tau // GRP
        t = tau % GRP
        drel_all[d, g, p, t] = (dd - w * P).astype(np.float16)

    attn_f = np.asarray(attn_e, np.float32).reshape(HF)
    attn_sel = np.zeros((HF, N_HEADS), dtype=np.float16)
    attn_sel[np.arange(HF), np.arange(HF) // OUT_FEATS] = \
        (0.4 * attn_f).astype(np.float16)

    def wu(W):
        aW = attn_f[:, None] * np.asarray(W, np.float32)       # [HF, IN]
        return np.ascontiguousarray(
            0.6 * aW.reshape(N_HEADS, OUT_FEATS, IN_FEATS).sum(1).T
        ).astype(np.float16)                                   # [IN, 4]

    cst = {
        "wsrcT": np.ascontiguousarray(
            np.asarray(W_src, np.float32).T).astype(np.float16),
        "wdstT": np.ascontiguousarray(
            np.asarray(W_dst, np.float32).T).astype(np.float16),
        "wresT": np.ascontiguousarray(
            np.asarray(W_res, np.float32).T).astype(np.float16),
        "attn_sel": attn_sel,
        "wu_src": wu(W_src),
        "wu_dst": wu(W_dst),
        "iota_row": np.tile(np.arange(P, dtype=np.float16)[None, :], (P, 1)),
    }

    gpp = (n_groups + NSPLIT - 1) // NSPLIT
    in_maps = []
    for d in range(M):
        m = dict(cst)
        # feature-major gathered features: per group [se 512 | de 512]
        fseT = feat16T[:, se_ids[d]]                    # [128, n_slots]
        fdeT = feat16T[:, de_ids[d]]
        sede = np.empty((P, n_groups, 2 * GRP * P), dtype=np.float16)
        sede[:, :, 0:GRP * P] = fseT.reshape(P, n_groups, GRP * P)
        sede[:, :, GRP * P:] = fdeT.reshape(P, n_groups, GRP * P)
        for i in range(NSPLIT):
            g0 = gpp * i
            g1 = min(gpp * (i + 1), n_groups)
            m[f"sedeT{i}"] = np.ascontiguousarray(
                sede[:, g0:g1, :].reshape(P, -1))
        fwin = np.zeros((OUT_ROWS, IN_FEATS), dtype=np.float16)
        fwin[:NLOC] = feat16[d * NLOC:(d + 1) * NLOC]
        m["fwinT"] = np.ascontiguousarray(fwin.T)
        m["drel"] = np.ascontiguousarray(
            drel_all[d].transpose(1, 0, 2).reshape(P, n_groups * GRP)).astype(np.float32)
        in_maps.append(m)
    return K, n_groups, in_maps


def kernel(feat, W_src, b_src, W_dst, b_dst, attn_e, W_res, b_res, src, dst,
           _trace=False, _trace_kwargs=None):
    K, n_groups, in_maps = _preprocess(feat, W_src, b_src, W_dst, b_dst,
                                       attn_e, W_res, b_res, src, dst)
    key = (K, n_groups)
    if key not in _prog_cache:
        _prog_cache[key] = _build_program(K, n_groups)
    nc = _prog_cache[key]

    kw = {}
    if _trace:
        kw = dict(trace=True, trace_kwargs=_trace_kwargs or {})
    res = run_bass_kernel_spmd(nc, in_maps, core_ids=list(range(M)), **kw)
    outs = [res.results[d]["out_d"][:NLOC] for d in range(M)]
    full = np.concatenate(outs, axis=0).reshape(N_NODES, N_HEADS, OUT_FEATS)
    kernel._last_results = res
    kernel._last_cfg = (K, n_groups)
    return full
